# revision 12
# baseline (speedup 1.0000x reference)
# Deformable Conv2d (B=4, C=CO=64, H=W=192, K=3, pad=1) on 8 Trainium2 NeuronCores.
#
# Strategy (data-parallel over B x half-image, 8 shards):
#   out[o,px] = sum_k sum_{c} wk[o,c,k] * bilinear_sample(x, px + base_k + (dy,dx)_k)
# Bilinear sampling with |d|<T/2 is an exact T-tap separable "hat filter" over a
# FIXED stencil around the base tap:  w(j) = relu(1 - |d - j|).  So
#   out[o,px] = sum_k sum_{u,v in taps} wy_ku[px]*wx_kv[px] * r_k[o, px+(by+u, bx+v)]
# where r_k = W_k @ x are 1x1-conv response planes.  On-device:
#   - PE computes, per 128-pixel tile (2 rows x 64 cols), all shifted r-slices
#     group-by-(sy,sx) as data-stationary matmuls: lhsT = x-slab slice (the shift
#     is a free-dim AP offset -> no data movement), rhs = stacked W_k. Out -> PSUM.
#   - The offset conv (18 channels) is 9 more tiny matmuls per tile -> PSUM.
#   - ACT builds the hat weights from the offsets; DVE combines with
#     scalar_tensor_tensor FMAs (per-partition scalars = per-pixel weights),
#     reading r straight from PSUM, accumulating out[px, o] in SBUF.
#   - Output DMAs out flat; the host wrapper restores [B, CO, H, W] layout.
import os
import numpy as np

B, C, CO, H, W = 4, 64, 64, 192, 192
K, PAD, KK = 3, 1, 9
N_CORES = 8
HALVES = N_CORES // B            # 2 half-images per batch sample
ROWS = H // HALVES               # 96 rows per core
HALO = 3                         # row halo each side (covers 5-tap reach)
PADC = 3                         # col zero-pad each side
WP = W + 2 * PADC                # 198
RSLAB = ROWS + 2 * HALO          # 102
TAPS = int(os.environ.get("DFC_TAPS", "3"))       # 3 or 5 hat taps per axis
REPS = int(os.environ.get("DFC_REPS", "1"))       # kernel-body repetitions (timing)
TR = (TAPS - 1) // 2
CB = 3                           # col blocks of 64 per row-pair
TILE_ROWS = ROWS // 2            # 48 row-pairs
N_TILES = TILE_ROWS * CB         # 144

# channel permutation of the offset conv (faithful to reference's reshape/
# transpose dance): new ch j<9 -> dy_j, j>=9 -> dx_{j-9}
DYPERM = [0, 4, 8, 12, 16, 3, 7, 11, 15]
DXPERM = [2, 6, 10, 14, 1, 5, 9, 13, 17]

BASE = [(k // 3 - 1, k % 3 - 1) for k in range(KK)]  # (by, bx) per k

# (sy, sx) groups: absolute shifts, with the k's whose hat window contains them
SHIFTS = []
for sy in range(-1 - TR, 2 + TR):
    for sx in range(-1 - TR, 2 + TR):
        ks = [k for k in range(KK)
              if abs(sy - BASE[k][0]) <= TR and abs(sx - BASE[k][1]) <= TR]
        if ks:
            SHIFTS.append((sy, sx, ks))
MAX_GROUP_K = 8  # keep matmul N = nk*64 <= 512 (one PSUM bank)

_CACHE = {}


def _build_program():
    import concourse.bacc as bacc
    import concourse.mybir as mybir
    from concourse import tile

    f32 = mybir.dt.float32
    bf16 = mybir.dt.bfloat16
    MUL = mybir.AluOpType.mult
    ADD = mybir.AluOpType.add
    AF = mybir.ActivationFunctionType

    nc = bacc.Bacc("TRN2", num_devices=N_CORES)
    xslab_d = nc.dram_tensor("xslab", [C + 1, RSLAB, WP], bf16, kind="ExternalInput")
    woffb_d = nc.dram_tensor("woffb", [C + 1, KK * 2 * KK], bf16, kind="ExternalInput")
    # wstack: concatenated [64, nk*64] blocks, one per (sy,sx) group (split to <=MAX_GROUP_K)
    groups = []
    for sy, sx, ks in SHIFTS:
        for i in range(0, len(ks), MAX_GROUP_K):
            groups.append((sy, sx, ks[i:i + MAX_GROUP_K]))
    wtot = sum(len(ks) for _, _, ks in groups) * CO
    wstack_d = nc.dram_tensor("wstack", [C, wtot], bf16, kind="ExternalInput")
    out_d = nc.dram_tensor("out", [N_TILES * 128, CO], f32, kind="ExternalOutput")

    NW = KK * TAPS  # columns of WY / WX

    with tile.TileContext(nc) as tc:
        with (
            tc.tile_pool(name="slab", bufs=1) as slab_pool,
            tc.tile_pool(name="consts", bufs=1) as const_pool,
            tc.tile_pool(name="wts", bufs=3) as wts_pool,
            tc.tile_pool(name="acc", bufs=3) as acc_pool,
            tc.tile_pool(name="psum", bufs=4, space="PSUM") as psum_pool,
            tc.tile_pool(name="ppsum", bufs=2, space="PSUM") as ppsum_pool,
        ):
            # per-partition constants for activation bias operands
            cvals = sorted({float(-(ui - TR)) for ui in range(TAPS)} | {1.0})
            cmap = {}
            for ci, v in enumerate(cvals):
                ct = const_pool.tile([128, 1], f32, tag=f"c{ci}",
                                     name=f"const{ci}")
                nc.vector.memset(ct[:, :], v)
                cmap[v] = ct

            xsb = slab_pool.tile([C + 1, RSLAB, WP], bf16)
            nc.sync.dma_start(xsb[:, :, :], xslab_d.ap()[:, :, :])
            woffb = const_pool.tile([C + 1, KK * 2 * KK], bf16)
            nc.sync.dma_start(woffb[:, :], woffb_d.ap()[:, :])
            wstack = const_pool.tile([C, wtot], bf16)
            nc.sync.dma_start(wstack[:, :], wstack_d.ap()[:, :])

            for rep in range(REPS):
              for hh in range(TILE_ROWS):
                for cb in range(CB):
                    t_idx = hh * CB + cb
                    r0 = 2 * hh + HALO          # slab row of tile's first row
                    c0 = PADC + cb * 64         # slab col of tile's first col

                    def xs(row, sy, sx, parts=C):
                        return xsb[0:parts, r0 + row + sy, c0 + sx:c0 + sx + 64]

                    # ---- offset conv: p[px, 18] ----
                    p_ps = ppsum_pool.tile([128, 2 * KK], f32, tag="p")
                    for row in range(2):
                        for k in range(KK):
                            by, bx = BASE[k]
                            nc.tensor.matmul(
                                p_ps[row * 64:(row + 1) * 64, :],
                                xs(row, by, bx, C + 1),
                                woffb[:, k * 18:(k + 1) * 18],
                                start=(k == 0), stop=(k == KK - 1),
                            )

                    # ---- hat weights: WY/WX [128, KK*TAPS], then products WYX ----
                    wy = wts_pool.tile([128, NW], f32, tag="wy")
                    wx = wts_pool.tile([128, NW], f32, tag="wx")
                    tmp = wts_pool.tile([128, KK], f32, tag="tmp")
                    for ax, wt in ((0, wy), (1, wx)):
                        dslice = p_ps[:, ax * KK:(ax + 1) * KK]
                        for ui in range(TAPS):
                            u = ui - TR
                            nc.scalar.activation(tmp[:, :], dslice, AF.Abs,
                                                 bias=cmap[float(-u)][:, :])
                            nc.scalar.activation(
                                wt[:, ui::TAPS], tmp[:, :], AF.Relu,
                                scale=-1.0, bias=cmap[1.0][:, :])
                    # products: WYX[:, k*T*T + ui*T + vi] = WY[:,k*T? strided]
                    wyx = wts_pool.tile([128, KK * TAPS * TAPS], f32, tag="wyx")
                    for ui in range(TAPS):
                        for vi in range(TAPS):
                            # cols k*T*T + ui*T + vi for k in 0..8
                            nc.vector.tensor_tensor(
                                wyx[:, ui * TAPS + vi::TAPS * TAPS],
                                wy[:, ui::TAPS],
                                wx[:, vi::TAPS],
                                MUL)

                    # ---- shifted response groups + modulated accumulation ----
                    accs = [acc_pool.tile([128, CO], f32, tag=f"acc{a}",
                                          name=f"acc{a}_{t_idx}")
                            for a in range(4)]
                    acc_started = [False] * 4
                    woff = 0
                    term = 0
                    for (sy, sx, ks) in groups:
                        nk = len(ks)
                        r_ps = psum_pool.tile([128, nk * CO], f32, tag="r")
                        for row in range(2):
                            nc.tensor.matmul(
                                r_ps[row * 64:(row + 1) * 64, :],
                                xs(row, sy, sx),
                                wstack[:, woff:woff + nk * CO],
                                start=True, stop=True)
                        woff += nk * CO
                        for j, k in enumerate(ks):
                            ui = sy - BASE[k][0] + TR
                            vi = sx - BASE[k][1] + TR
                            sc = wyx[:, k * TAPS * TAPS + ui * TAPS + vi
                                     :k * TAPS * TAPS + ui * TAPS + vi + 1]
                            a = term % 4
                            term += 1
                            rsl = r_ps[:, j * CO:(j + 1) * CO]
                            if not acc_started[a]:
                                nc.scalar.activation(accs[a][:, :], rsl, AF.Copy,
                                                     scale=sc)
                                acc_started[a] = True
                            else:
                                nc.vector.scalar_tensor_tensor(
                                    accs[a][:, :], rsl, sc, accs[a][:, :],
                                    MUL, ADD)
                    # acc0 += acc1; acc2 += acc3; acc0 += acc2
                    nc.vector.tensor_tensor(accs[0][:, :], accs[0][:, :],
                                            accs[1][:, :], ADD)
                    nc.vector.tensor_tensor(accs[2][:, :], accs[2][:, :],
                                            accs[3][:, :], ADD)
                    osum = acc_pool.tile([128, CO], f32, tag="osum")
                    nc.vector.tensor_tensor(osum[:, :], accs[0][:, :],
                                            accs[2][:, :], ADD)
                    nc.sync.dma_start(
                        out_d.ap()[t_idx * 128:(t_idx + 1) * 128, :], osum[:, :])

    nc.compile()
    return nc


def _prep_weights(w_deform, w_offset, b_offset):
    # offset conv weights with output channels permuted to [dy(9), dx(9)],
    # bias folded as contract-row 64 on the center (k==4) tap block.
    perm = DYPERM + DXPERM
    wo = w_offset[perm]          # [18, C, 3, 3]
    bo = b_offset[perm]          # [18]
    woffb = np.zeros((C + 1, KK * 18), np.float32)
    for k in range(KK):
        ky, kx = k // 3, k % 3
        woffb[:C, k * 18:(k + 1) * 18] = wo[:, :, ky, kx].T
    woffb[C, 4 * 18:5 * 18] = bo
    # stacked deform weights per (sy,sx) group
    blocks = []
    for sy, sx, ks in SHIFTS:
        for i in range(0, len(ks), MAX_GROUP_K):
            for k in ks[i:i + MAX_GROUP_K]:
                blocks.append(w_deform[:, :, k // 3, k % 3].T)  # [C, CO]
    wstack = np.concatenate(blocks, axis=1).astype(np.float32)
    import ml_dtypes
    return woffb.astype(ml_dtypes.bfloat16), wstack.astype(ml_dtypes.bfloat16)


def kernel(x, w_deform, w_offset, b_offset):
    from concourse.bass_utils import run_bass_kernel_spmd

    if "nc" not in _CACHE:
        _CACHE["nc"] = _build_program()
    nc = _CACHE["nc"]

    woffb, wstack = _prep_weights(
        np.asarray(w_deform, np.float32),
        np.asarray(w_offset, np.float32),
        np.asarray(b_offset, np.float32))

    x = np.asarray(x, np.float32)
    in_maps = []
    for core in range(N_CORES):
        b, half = core // HALVES, core % HALVES
        import ml_dtypes
        slab = np.zeros((C + 1, RSLAB, WP), ml_dtypes.bfloat16)
        slab[C] = 1.0
        r_lo = half * ROWS - HALO
        r_hi = half * ROWS + ROWS + HALO
        src_lo, src_hi = max(r_lo, 0), min(r_hi, H)
        slab[:C, src_lo - r_lo:src_hi - r_lo, PADC:PADC + W] = \
            x[b, :, src_lo:src_hi, :].astype(ml_dtypes.bfloat16)
        in_maps.append({"xslab": slab, "woffb": woffb, "wstack": wstack})

    res = run_bass_kernel_spmd(nc, in_maps, core_ids=list(range(N_CORES)))

    out = np.empty((B, CO, H, W), np.float32)
    for core in range(N_CORES):
        b, half = core // HALVES, core % HALVES
        o = res.results[core]["out"]          # [144*128, 64]
        o = o.reshape(TILE_ROWS, CB, 2, 64, CO)
        o = o.transpose(4, 0, 2, 1, 3).reshape(CO, ROWS, W)
        out[b, :, half * ROWS:(half + 1) * ROWS, :] = o
    return out


if __name__ == "__main__":
    xs = {k: np.load(f"/tmp/in_{k}.npy") for k in
          ("x", "w_deform", "w_offset", "b_offset")}
    got = kernel(**xs)
    exp = np.load("/tmp/expected.npy")
    err = np.abs(got - exp)
    rel = np.linalg.norm(got - exp) / np.linalg.norm(exp)
    print(f"absmax={err.max():.6f} rel-l2={rel:.3e}")


# revision 15
# speedup vs baseline: 803.5830x; 803.5830x over previous
# Deformable Conv2d (B=4, C=CO=64, H=W=192, K=3, pad=1) on 8 Trainium2 NeuronCores.
#
# Strategy (data-parallel over B x half-image, 8 shards):
#   out[o,px] = sum_k sum_{c} wk[o,c,k] * bilinear_sample(x, px + base_k + (dy,dx)_k)
# Bilinear sampling with |d|<T/2 is an exact T-tap separable "hat filter" over a
# FIXED stencil around the base tap:  w(j) = relu(1 - |d - j|).  So
#   out[o,px] = sum_k sum_{u,v in taps} wy_ku[px]*wx_kv[px] * r_k[o, px+(by+u, bx+v)]
# where r_k = W_k @ x are 1x1-conv response planes.  On-device:
#   - PE computes, per 128-pixel tile (2 rows x 64 cols), all shifted r-slices
#     group-by-(sy,sx) as data-stationary matmuls: lhsT = x-slab slice (the shift
#     is a free-dim AP offset -> no data movement), rhs = stacked W_k. Out -> PSUM.
#   - The offset conv (18 channels) is 9 more tiny matmuls per tile -> PSUM.
#   - ACT builds the hat weights from the offsets; DVE combines with
#     scalar_tensor_tensor FMAs (per-partition scalars = per-pixel weights),
#     reading r straight from PSUM, accumulating out[px, o] in SBUF.
#   - Output DMAs out flat; the host wrapper restores [B, CO, H, W] layout.
import os
import numpy as np

B, C, CO, H, W = 4, 64, 64, 192, 192
K, PAD, KK = 3, 1, 9
N_CORES = 8
HALVES = N_CORES // B            # 2 half-images per batch sample
ROWS = H // HALVES               # 96 rows per core
HALO = 3                         # row halo each side (covers 5-tap reach)
PADC = 3                         # col zero-pad each side
WP = W + 2 * PADC                # 198
RSLAB = ROWS + 2 * HALO          # 102
TAPS = int(os.environ.get("DFC_TAPS", "3"))       # 3 or 5 hat taps per axis
LOOPT = int(os.environ.get("DFC_LOOPT", "1"))    # hw-loop trip count (timing)
TR = (TAPS - 1) // 2
CB = 3                           # col blocks of 64 per row-pair
TILE_ROWS = ROWS // 2            # 48 row-pairs
N_TILES = TILE_ROWS * CB         # 144

# channel permutation of the offset conv (faithful to reference's reshape/
# transpose dance): new ch j<9 -> dy_j, j>=9 -> dx_{j-9}
DYPERM = [0, 4, 8, 12, 16, 3, 7, 11, 15]
DXPERM = [2, 6, 10, 14, 1, 5, 9, 13, 17]

BASE = [(k // 3 - 1, k % 3 - 1) for k in range(KK)]  # (by, bx) per k

# (sy, sx) groups: absolute shifts, with the k's whose hat window contains them
SHIFTS = []
for sy in range(-1 - TR, 2 + TR):
    for sx in range(-1 - TR, 2 + TR):
        ks = [k for k in range(KK)
              if abs(sy - BASE[k][0]) <= TR and abs(sx - BASE[k][1]) <= TR]
        if ks:
            SHIFTS.append((sy, sx, ks))
MAX_GROUP_K = 8  # keep matmul N = nk*64 <= 512 (one PSUM bank)

_CACHE = {}


def _build_program():
    import concourse.bacc as bacc
    import concourse.mybir as mybir
    from concourse import tile

    f32 = mybir.dt.float32
    bf16 = mybir.dt.bfloat16
    MUL = mybir.AluOpType.mult
    ADD = mybir.AluOpType.add
    AF = mybir.ActivationFunctionType

    nc = bacc.Bacc("TRN2", num_devices=N_CORES)
    xslab_d = nc.dram_tensor("xslab", [C + 1, RSLAB, WP], bf16, kind="ExternalInput")
    woffb_d = nc.dram_tensor("woffb", [C + 1, KK * 2 * KK], bf16, kind="ExternalInput")
    # wstack: concatenated [64, nk*64] blocks, one per (sy,sx) group (split to <=MAX_GROUP_K)
    groups = []
    for sy, sx, ks in SHIFTS:
        for i in range(0, len(ks), MAX_GROUP_K):
            groups.append((sy, sx, ks[i:i + MAX_GROUP_K]))
    wtot = sum(len(ks) for _, _, ks in groups) * CO
    wstack_d = nc.dram_tensor("wstack", [C, wtot], bf16, kind="ExternalInput")
    out_d = nc.dram_tensor("out", [N_TILES * 128, CO], f32, kind="ExternalOutput")

    NW = KK * TAPS  # columns of WY / WX

    with tile.TileContext(nc) as tc:
        with (
            tc.tile_pool(name="slab", bufs=1) as slab_pool,
            tc.tile_pool(name="consts", bufs=1) as const_pool,
            tc.tile_pool(name="wts", bufs=3) as wts_pool,
            tc.tile_pool(name="acc", bufs=3) as acc_pool,
            tc.tile_pool(name="psum", bufs=4, space="PSUM") as psum_pool,
            tc.tile_pool(name="ppsum", bufs=2, space="PSUM") as ppsum_pool,
        ):
            # per-partition constants for activation bias operands
            cvals = sorted({float(-(ui - TR)) for ui in range(TAPS)} | {1.0})
            cmap = {}
            for ci, v in enumerate(cvals):
                ct = const_pool.tile([128, 1], f32, tag=f"c{ci}",
                                     name=f"const{ci}")
                nc.vector.memset(ct[:, :], v)
                cmap[v] = ct

            xsb = slab_pool.tile([C + 1, RSLAB, WP], bf16)
            nc.sync.dma_start(xsb[:, :, :], xslab_d.ap()[:, :, :])
            woffb = const_pool.tile([C + 1, KK * 2 * KK], bf16)
            nc.sync.dma_start(woffb[:, :], woffb_d.ap()[:, :])
            wstack = const_pool.tile([C, wtot], bf16)
            nc.sync.dma_start(wstack[:, :], wstack_d.ap()[:, :])

            import contextlib
            loop_cm = tc.For_i(0, LOOPT, 1) if LOOPT > 1 else contextlib.nullcontext()
            with loop_cm:
              for hh in range(TILE_ROWS):
                for cb in range(CB):
                    t_idx = hh * CB + cb
                    r0 = 2 * hh + HALO          # slab row of tile's first row
                    c0 = PADC + cb * 64         # slab col of tile's first col

                    def xs(row, sy, sx, parts=C):
                        return xsb[0:parts, r0 + row + sy, c0 + sx:c0 + sx + 64]

                    # ---- offset conv: p[px, 18] ----
                    p_ps = ppsum_pool.tile([128, 2 * KK], f32, tag="p")
                    for row in range(2):
                        for k in range(KK):
                            by, bx = BASE[k]
                            nc.tensor.matmul(
                                p_ps[row * 64:(row + 1) * 64, :],
                                xs(row, by, bx, C + 1),
                                woffb[:, k * 18:(k + 1) * 18],
                                start=(k == 0), stop=(k == KK - 1),
                            )

                    # ---- hat weights: WY/WX [128, KK*TAPS], then products WYX ----
                    wy = wts_pool.tile([128, NW], f32, tag="wy")
                    wx = wts_pool.tile([128, NW], f32, tag="wx")
                    tmp = wts_pool.tile([128, KK], f32, tag="tmp")
                    for ax, wt in ((0, wy), (1, wx)):
                        dslice = p_ps[:, ax * KK:(ax + 1) * KK]
                        for ui in range(TAPS):
                            u = ui - TR
                            nc.scalar.activation(tmp[:, :], dslice, AF.Abs,
                                                 bias=cmap[float(-u)][:, :])
                            nc.scalar.activation(
                                wt[:, ui::TAPS], tmp[:, :], AF.Relu,
                                scale=-1.0, bias=cmap[1.0][:, :])
                    # products: WYX[:, k*T*T + ui*T + vi] = WY[:,k*T? strided]
                    wyx = wts_pool.tile([128, KK * TAPS * TAPS], f32, tag="wyx")
                    for ui in range(TAPS):
                        for vi in range(TAPS):
                            # cols k*T*T + ui*T + vi for k in 0..8
                            nc.vector.tensor_tensor(
                                wyx[:, ui * TAPS + vi::TAPS * TAPS],
                                wy[:, ui::TAPS],
                                wx[:, vi::TAPS],
                                MUL)

                    # ---- shifted response groups + modulated accumulation ----
                    accs = [acc_pool.tile([128, CO], f32, tag=f"acc{a}",
                                          name=f"acc{a}_{t_idx}")
                            for a in range(4)]
                    acc_started = [False] * 4
                    woff = 0
                    term = 0
                    for (sy, sx, ks) in groups:
                        nk = len(ks)
                        r_ps = psum_pool.tile([128, nk * CO], f32, tag="r")
                        for row in range(2):
                            nc.tensor.matmul(
                                r_ps[row * 64:(row + 1) * 64, :],
                                xs(row, sy, sx),
                                wstack[:, woff:woff + nk * CO],
                                start=True, stop=True)
                        woff += nk * CO
                        for j, k in enumerate(ks):
                            ui = sy - BASE[k][0] + TR
                            vi = sx - BASE[k][1] + TR
                            sc = wyx[:, k * TAPS * TAPS + ui * TAPS + vi
                                     :k * TAPS * TAPS + ui * TAPS + vi + 1]
                            a = term % 4
                            term += 1
                            rsl = r_ps[:, j * CO:(j + 1) * CO]
                            if not acc_started[a]:
                                nc.scalar.activation(accs[a][:, :], rsl, AF.Copy,
                                                     scale=sc)
                                acc_started[a] = True
                            else:
                                nc.vector.scalar_tensor_tensor(
                                    accs[a][:, :], rsl, sc, accs[a][:, :],
                                    MUL, ADD)
                    # acc0 += acc1; acc2 += acc3; acc0 += acc2
                    nc.vector.tensor_tensor(accs[0][:, :], accs[0][:, :],
                                            accs[1][:, :], ADD)
                    nc.vector.tensor_tensor(accs[2][:, :], accs[2][:, :],
                                            accs[3][:, :], ADD)
                    osum = acc_pool.tile([128, CO], f32, tag="osum")
                    nc.vector.tensor_tensor(osum[:, :], accs[0][:, :],
                                            accs[2][:, :], ADD)
                    nc.sync.dma_start(
                        out_d.ap()[t_idx * 128:(t_idx + 1) * 128, :], osum[:, :])

    nc.compile()
    return nc


def _prep_weights(w_deform, w_offset, b_offset):
    # offset conv weights with output channels permuted to [dy(9), dx(9)],
    # bias folded as contract-row 64 on the center (k==4) tap block.
    perm = DYPERM + DXPERM
    wo = w_offset[perm]          # [18, C, 3, 3]
    bo = b_offset[perm]          # [18]
    woffb = np.zeros((C + 1, KK * 18), np.float32)
    for k in range(KK):
        ky, kx = k // 3, k % 3
        woffb[:C, k * 18:(k + 1) * 18] = wo[:, :, ky, kx].T
    woffb[C, 4 * 18:5 * 18] = bo
    # stacked deform weights per (sy,sx) group
    blocks = []
    for sy, sx, ks in SHIFTS:
        for i in range(0, len(ks), MAX_GROUP_K):
            for k in ks[i:i + MAX_GROUP_K]:
                blocks.append(w_deform[:, :, k // 3, k % 3].T)  # [C, CO]
    wstack = np.concatenate(blocks, axis=1).astype(np.float32)
    import ml_dtypes
    return woffb.astype(ml_dtypes.bfloat16), wstack.astype(ml_dtypes.bfloat16)


def kernel(x, w_deform, w_offset, b_offset):
    from concourse.bass_utils import run_bass_kernel_spmd

    if "nc" not in _CACHE:
        _CACHE["nc"] = _build_program()
    nc = _CACHE["nc"]

    woffb, wstack = _prep_weights(
        np.asarray(w_deform, np.float32),
        np.asarray(w_offset, np.float32),
        np.asarray(b_offset, np.float32))

    x = np.asarray(x, np.float32)
    in_maps = []
    for core in range(N_CORES):
        b, half = core // HALVES, core % HALVES
        import ml_dtypes
        slab = np.zeros((C + 1, RSLAB, WP), ml_dtypes.bfloat16)
        slab[C] = 1.0
        r_lo = half * ROWS - HALO
        r_hi = half * ROWS + ROWS + HALO
        src_lo, src_hi = max(r_lo, 0), min(r_hi, H)
        slab[:C, src_lo - r_lo:src_hi - r_lo, PADC:PADC + W] = \
            x[b, :, src_lo:src_hi, :].astype(ml_dtypes.bfloat16)
        in_maps.append({"xslab": slab, "woffb": woffb, "wstack": wstack})

    res = run_bass_kernel_spmd(nc, in_maps, core_ids=list(range(N_CORES)))

    out = np.empty((B, CO, H, W), np.float32)
    for core in range(N_CORES):
        b, half = core // HALVES, core % HALVES
        o = res.results[core]["out"]          # [144*128, 64]
        o = o.reshape(TILE_ROWS, CB, 2, 64, CO)
        o = o.transpose(4, 0, 2, 1, 3).reshape(CO, ROWS, W)
        out[b, :, half * ROWS:(half + 1) * ROWS, :] = o
    return out


if __name__ == "__main__":
    xs = {k: np.load(f"/tmp/in_{k}.npy") for k in
          ("x", "w_deform", "w_offset", "b_offset")}
    got = kernel(**xs)
    exp = np.load("/tmp/expected.npy")
    err = np.abs(got - exp)
    rel = np.linalg.norm(got - exp) / np.linalg.norm(exp)
    print(f"absmax={err.max():.6f} rel-l2={rel:.3e}")
